# revision 1
# baseline (speedup 1.0000x reference)
"""ClsbdCRF message-passing kernel for 8 Trainium2 NeuronCores.

Sharding: core i handles batch b = i//2 and image-row half i%2 (64 output
rows each, with span-2 halos sliced host-side).  Per-core SBUF layout puts
W=128 on partitions and (C, H) on the free dimension, so the 5x5 stencil
becomes partition-offset (dy) + free-offset (dx) access patterns.

Math per core (fp32):
  pl   = 1 - ent/ln(C),  ent = -sum_c x ln(x+eps)
  xp   = x * pl
  g1_d = exp(-0.5 * ||f(x) - f(x+d)||^2)            (12 taps + mirrors + center)
  g2_t = ring-max propagation of unfolded clsbd map  (24 taps + center=0)
  w_neg_t = 2*g1_t - ln(g2_t+eps)          (x5 at the end)
  w_pos_t = ln(1 - g2_t + eps)             (x-5 at the end)
  msg[c,h,w] = sum_t w_t[h,w] * xp[c, (h,w)+t]

Boundary handling: H is zero-padded host-side (feats big-padded so the
pairwise gaussian underflows to exactly 0 out of image); W taps use
restricted partition ranges with pre-zeroed destination tiles.
"""

import math

import numpy as np

B, C, H, W, D = 4, 21, 128, 128, 5
SPAN = 2
EPS = 1e-5
HP = 64          # output rows per core
HE = HP + 4      # input / clsbd row extent (halo 2 each side)
FE = HP + 8      # feats row extent (halo 4 each side)
BIGPAD = 1000.0  # feats pad value; (BIGPAD-x)^2 makes exp() underflow to 0
COMPAT_PAIR = 10.0
COMPAT_CLSBD = 5.0

RING1 = [(-1, -1), (-1, 0), (-1, 1), (0, -1), (0, 1), (1, -1), (1, 0), (1, 1)]
RING2 = [(-2, -2), (-2, -1), (-2, 0), (-2, 1), (-2, 2), (-1, -2), (-1, 2),
         (0, -2), (0, 2), (1, -2), (1, 2), (2, -2), (2, -1), (2, 0), (2, 1),
         (2, 2)]
EXP1 = [0, 0, 1, 2, 2, 0, 2, 3, 4, 5, 7, 5, 5, 6, 7, 7]
EXP2 = [0, 1, 1, 1, 2, 3, 4, 3, 4, 3, 4, 5, 6, 6, 6, 7]
# taps whose pairwise gaussian is computed directly; mirrors are shifted reads
DIRTAPS = [(dx, dy) for dx in range(-SPAN, SPAN + 1)
           for dy in range(-SPAN, SPAN + 1) if (dx, dy) > (0, 0)]
ALLTAPS = [(dx, dy) for dx in range(-SPAN, SPAN + 1)
           for dy in range(-SPAN, SPAN + 1)]

GP_NTAPS = 8
_cache = {}


def _wrange(dy):
    return max(0, -dy), W - max(0, dy)


def _build():
    import concourse.bacc as bacc
    import concourse.mybir as mybir
    from concourse.tile import TileContext

    f32 = mybir.dt.float32
    Act = mybir.ActivationFunctionType
    Alu = mybir.AluOpType

    nc = bacc.Bacc()
    x_d = nc.declare_dram_parameter("x", [W, C, HE], f32, isOutput=False)
    f_d = nc.declare_dram_parameter("f", [W, D, FE], f32, isOutput=False)
    s_d = nc.declare_dram_parameter("s", [W, HE], f32, isOutput=False)
    o_d = nc.declare_dram_parameter("out", [2, W, C, HP], f32, isOutput=True)

    DYS = [-2, -1, 1, 2]

    # Pre-TileContext constants, covered by an all-engine barrier (same
    # pattern Bass.__init__ uses) so consumers never need a sync wait.
    def _const_sbuf(name, shape, val):
        t = nc.alloc_sbuf_tensor(name, shape, f32)
        nc.gpsimd.memset(t.ap(), val)
        return t.ap()

    zt = _const_sbuf("zt_const", [W, C, HE], 0.0)
    bpad = _const_sbuf("bpad_const", [W, D, FE], BIGPAD)
    b_eps = _const_sbuf("b_eps", [W, 1], EPS)
    b_ln2 = _const_sbuf("b_ln2", [W, 1], math.log(2.0))
    b_1eps = _const_sbuf("b_1eps", [W, 1], 1.0 + EPS)
    nc.const_aps.aps[(f32, EPS)] = b_eps

    # partition-shift matrices: S_dy[k, m] = 1 iff k = m + dy, so
    # (S_dy^T @ x)[m] = x[m+dy] with zero rows outside [0, W) — PE does
    # the partition shift straight into PSUM, no DMA descriptor storms.
    s_mat = {}
    for dy in (-2, -1, 1, 2):
        t = nc.alloc_sbuf_tensor(f"shift_{dy}", [W, W], f32)
        nc.gpsimd.memset(t.ap(), 0.0)
        nc.gpsimd.affine_select(
            out=t.ap(), in_=t.ap(), compare_op=mybir.AluOpType.not_equal,
            fill=1.0, base=-dy, pattern=[[-1, W]], channel_multiplier=1)
        s_mat[dy] = t.ap()
    nc.all_engine_barrier()

    # taps owned end-to-end by GpSimd (ready earliest: dy=0 needs no
    # shifted xp at all). DVE keeps the rest.
    GP_TAPS = [(1, 0), (2, 0), (-1, 0), (-2, 0)][:GP_NTAPS]

    with TileContext(nc) as tc:
        with (
            tc.tile_pool(name="io", bufs=1) as io,
            tc.tile_pool(name="g1p", bufs=1) as g1p,
            tc.tile_pool(name="g2p", bufs=1) as g2p,
            tc.tile_pool(name="wp", bufs=1) as wp,
            tc.tile_pool(name="lp", bufs=3) as lp,
            tc.tile_pool(name="mp", bufs=2) as mp,
            tc.tile_pool(name="sc", bufs=1) as sc,
            tc.tile_pool(name="scr", bufs=2) as scr,
            tc.tile_pool(name="dr", bufs=1, space="DRAM") as dr,
            tc.tile_pool(name="psp", bufs=2, space="PSUM") as psp,
        ):
            # ---- phase 0: all DRAM loads up front ----
            x_t = io.tile([W, C, HE], f32, tag="x")
            f_t = io.tile([W, D, FE], f32, tag="f")
            s_t = io.tile([W, HE], f32, tag="s")
            nc.sync.dma_start(out=x_t[:], in_=x_d[:])
            nc.sync.dma_start(out=f_t[:], in_=f_d[:])
            nc.sync.dma_start(out=s_t[:], in_=s_d[:])

            def _shift_load(pool, tag, shape, dram, padsrc, dy, eng=None):
                # dy-shifted copy straight from DRAM (one descriptor),
                # out-of-range partitions filled from a barrier-covered const
                eng = eng or nc.sync
                t = pool.tile(shape, f32, tag=tag)
                a, b = _wrange(dy)
                if a > 0:
                    eng.dma_start(out=t[:a], in_=padsrc[:a])
                if b < W:
                    eng.dma_start(out=t[b:], in_=padsrc[b:])
                eng.dma_start(out=t[a:b], in_=dram[a + dy:b + dy])
                return t

            f_s, s_s = {0: f_t}, {0: s_t}
            for dy in DYS:
                f_s[dy] = _shift_load(io, f"fs_{dy}", [W, D, FE], f_d, bpad,
                                      dy)
                s_s[dy] = _shift_load(io, f"ss_{dy}", [W, HE], s_d,
                                      zt[:, 0, :HE], dy)

            # ---- polarness ----
            lnx = sc.tile([W, C, HE], f32, tag="lnx")
            nc.scalar.activation(lnx[:], x_t[:], Act.Ln, bias=b_eps[:], scale=1.0)
            xl = sc.tile([W, C, HE], f32, tag="xl")
            nc.vector.tensor_mul(xl[:], x_t[:], lnx[:])
            ent = sc.tile([W, HE], f32, tag="ent")
            nc.vector.tensor_reduce(
                out=ent[:], in_=xl[:].rearrange("p c h -> p h c"),
                axis=mybir.AxisListType.X, op=Alu.add)
            pl = sc.tile([W, HE], f32, tag="pl")
            # ent holds sum_c x*ln(x+eps) = -entropy
            nc.scalar.activation(pl[:], ent[:], Act.Copy,
                                 bias=1.0, scale=1.0 / math.log(C))
            xp = io.tile([W, C, HE], f32, tag="xp")
            nc.vector.tensor_mul(
                xp[:], x_t[:], pl[:, None, :].broadcast_to((W, C, HE)))
            # xp dy-shifts: 3 matmuls per dy (<=512 fp32 moving-operand
            # cap; 512-f32 slices stay single-bank). 2 PSUM slots rotate
            # through the dy groups, so products must consume dy-major.
            xp_flat = xp[:].rearrange("p c h -> p (c h)")
            FSL = [(0, 512), (512, 1024), (1024, C * HE)]
            xp_s = {0: xp}
            for dy in (-1, 1, -2, 2):
                t = psp.tile([W, C, HE], f32, tag="xps")
                tf = t[:].rearrange("p c h -> p (c h)")
                for (n0, n1) in FSL:
                    nc.tensor.matmul(tf[:, n0:n1], s_mat[dy],
                                     xp_flat[:, n0:n1], start=True, stop=True)
                xp_s[dy] = t

            # ---- pairwise gaussian (12 direct taps; value stored = 2*g1) ----
            # BIGPAD-shifted feats make out-of-image taps underflow to 0.
            g1t = {}
            g1d = {}
            for (dx, dy) in DIRTAPS:
                g1 = g1p.tile([W, HE], f32, tag=f"g1_{dx}_{dy}")
                diff = scr.tile([W, D, HE], f32, tag="diff")
                nc.vector.tensor_sub(
                    diff[:], f_t[:, :, 2:2 + HE],
                    f_s[dy][:, :, 2 + dx:2 + dx + HE])
                sq = scr.tile([W, D, HE], f32, tag="sq")
                nc.scalar.square(sq[:], diff[:])
                ssum = scr.tile([W, HE], f32, tag="ssum")
                nc.vector.tensor_reduce(
                    out=ssum[:], in_=sq[:].rearrange("p d h -> p h d"),
                    axis=mybir.AxisListType.X, op=Alu.add)
                nc.scalar.activation(g1[:], ssum[:], Act.Exp,
                                     bias=b_ln2[:], scale=-0.5)
                g1t[(dx, dy)] = g1
                if dy != 0:
                    gd = dr.tile([W, HE], f32, tag=f"g1d_{dx}_{dy}")
                    nc.scalar.dma_start(out=gd[:], in_=g1[:])
                    g1d[(dx, dy)] = gd
            # dy-shifted mirror copies via DRAM roundtrip
            g1s = {}
            for (dx, dy) in DIRTAPS:
                if dy == 0:
                    g1s[(dx, dy)] = g1t[(dx, dy)]
                    continue
                a, b = _wrange(-dy)
                t = g1p.tile([W, HE], f32, tag=f"g1s_{dx}_{dy}")
                if a > 0:
                    nc.scalar.dma_start(out=t[:a], in_=zt[:a, 0, :HE])
                if b < W:
                    nc.scalar.dma_start(out=t[b:], in_=zt[b:, 0, :HE])
                nc.scalar.dma_start(out=t[a:b],
                                    in_=g1d[(dx, dy)][a - dy:b - dy])
                g1s[(dx, dy)] = t

            # ---- clsbd gaussian: ring max propagation ----
            tmp1 = [s_s[dy][:, 2 + dx:2 + dx + HP] for (dx, dy) in RING1]
            g2t = {t: tmp1[j] for j, t in enumerate(RING1)}
            for k, (dx, dy) in enumerate(RING2):
                t2 = g2p.tile([W, HP], f32, tag=f"t2_{k}")
                nc.vector.tensor_max(t2[:], tmp1[EXP1[k]], tmp1[EXP2[k]])
                nc.vector.tensor_max(
                    t2[:], t2[:], s_s[dy][:, 2 + dx:2 + dx + HP])
                g2t[(dx, dy)] = t2

            # ---- weights for all 24 taps ----
            NT = [t for t in ALLTAPS if t != (0, 0)]
            # GpSimd-owned taps first in emission so its chain starts early
            DYRANK = {0: -1, -1: 0, 1: 1, -2: 2, 2: 3}
            NT.sort(key=lambda t: (t not in GP_TAPS, DYRANK[t[1]]))
            wns, lnps = {}, {}
            for (dx, dy) in NT:
                g2 = g2t[(dx, dy)]
                g2ap = g2[:] if hasattr(g2, "tag") else g2
                lnn = lp.tile([W, HP], f32, tag="lnn")
                nc.scalar.activation(lnn[:], g2ap, Act.Ln, bias=b_eps[:],
                                     scale=1.0)
                lnp = wp.tile([W, HP], f32, tag=f"lnp_{dx}_{dy}")
                nc.scalar.activation(lnp[:], g2ap, Act.Ln,
                                     bias=b_1eps[:], scale=-1.0)
                wn = wp.tile([W, HP], f32, tag=f"wn_{dx}_{dy}")
                if (dx, dy) > (0, 0):
                    g1ap = g1t[(dx, dy)][:, 2:2 + HP]
                else:
                    g1ap = g1s[(-dx, -dy)][:, 2 + dx:2 + dx + HP]
                nc.vector.tensor_sub(wn[:], g1ap, lnn[:])
                wns[(dx, dy)] = wn
                lnps[(dx, dy)] = lnp

            # ---- products + accumulation ----
            accn = io.tile([W, C, HP], f32, tag="accn")
            accp = io.tile([W, C, HP], f32, tag="accp")
            accn2 = io.tile([W, C, HP], f32, tag="accn2")
            accp2 = io.tile([W, C, HP], f32, tag="accp2")
            xp_c = xp[:, :, 2:2 + HP]
            nc.vector.tensor_scalar_mul(accn[:], xp_c, 2.0 - math.log(EPS))
            nc.vector.tensor_scalar_mul(accp[:], xp_c, math.log(1.0 + EPS))
            gp_first = [True]
            for (dx, dy) in NT:
                wnb = wns[(dx, dy)][:, None, :].broadcast_to((W, C, HP))
                lpb = lnps[(dx, dy)][:, None, :].broadcast_to((W, C, HP))
                xpap = xp_s[dy][:, :, 2 + dx:2 + dx + HP]
                if (dx, dy) in GP_TAPS:
                    if gp_first[0]:
                        nc.gpsimd.tensor_mul(accn2[:], wnb, xpap)
                        nc.gpsimd.tensor_mul(accp2[:], lpb, xpap)
                        gp_first[0] = False
                    else:
                        tn = mp.tile([W, C, HP], f32, tag="tng")
                        nc.gpsimd.tensor_mul(tn[:], wnb, xpap)
                        nc.gpsimd.tensor_add(accn2[:], accn2[:], tn[:])
                        tp = mp.tile([W, C, HP], f32, tag="tpg")
                        nc.gpsimd.tensor_mul(tp[:], lpb, xpap)
                        nc.gpsimd.tensor_add(accp2[:], accp2[:], tp[:])
                else:
                    tn = mp.tile([W, C, HP], f32, tag="tn")
                    nc.vector.tensor_mul(tn[:], wnb, xpap)
                    nc.vector.tensor_add(accn[:], accn[:], tn[:])
                    tp = mp.tile([W, C, HP], f32, tag="tp")
                    nc.vector.tensor_mul(tp[:], lpb, xpap)
                    nc.vector.tensor_add(accp[:], accp[:], tp[:])

            nc.vector.tensor_add(accn[:], accn[:], accn2[:])
            nc.vector.tensor_add(accp[:], accp[:], accp2[:])
            nc.scalar.activation(accn[:], accn[:], Act.Copy,
                                 bias=0.0, scale=COMPAT_CLSBD)
            nc.scalar.activation(accp[:], accp[:], Act.Copy,
                                 bias=0.0, scale=-COMPAT_CLSBD)
            nc.sync.dma_start(out=o_d[0], in_=accn[:])
            nc.sync.dma_start(out=o_d[1], in_=accp[:])
    nc.finalize()
    return nc


_last_results = None


def kernel(input, feats, clsbd_feats, label=None, **_ignored):
    global _last_results
    from concourse.bass_utils import run_bass_kernel_spmd

    x = np.asarray(input, np.float32)
    f = np.asarray(feats, np.float32)
    s = np.asarray(clsbd_feats, np.float32)

    xpad = np.zeros((B, C, H + 4, W), np.float32)
    xpad[:, :, 2:2 + H] = x
    fpad = np.full((B, D, H + 8, W), BIGPAD, np.float32)
    fpad[:, :, 4:4 + H] = f
    spad = np.zeros((B, H + 4, W), np.float32)
    spad[:, 2:2 + H] = s[:, 0]

    in_maps = []
    for i in range(8):
        b, half = i // 2, i % 2
        h0 = half * HP
        in_maps.append({
            "x": np.ascontiguousarray(
                xpad[b, :, h0:h0 + HE].transpose(2, 0, 1)),
            "f": np.ascontiguousarray(
                fpad[b, :, h0:h0 + FE].transpose(2, 0, 1)),
            "s": np.ascontiguousarray(spad[b, h0:h0 + HE].transpose(1, 0)),
        })

    if "nc" not in _cache:
        _cache["nc"] = _build()
    res = run_bass_kernel_spmd(_cache["nc"], in_maps, list(range(8)))
    _last_results = res

    out = np.empty((2, B, C, H, W), np.float32)
    for i in range(8):
        b, half = i // 2, i % 2
        h0 = half * HP
        out[:, b, :, h0:h0 + HP] = res.results[i]["out"].transpose(0, 2, 3, 1)
    return out



# revision 14
# speedup vs baseline: 1.8928x; 1.8928x over previous
"""ClsbdCRF message passing on 8 NeuronCores — weight-shift formulation.

Core i handles batch i//2, image-row half i%2 (64 output rows + halo).
Layout: W=128 on partitions, (slots, C, H) on free dims, fp16 compute with
fp32 PSUM accumulation.

msg[p] = sum_t w_t[p] * xp[p + d_t] is re-associated as
u_t[p'] = w_t[p' - d_t] * xp[p']: weights are shifted in COMPACT
[W, 5, 64] per-dy space by PE matmuls whose stationary shift matrices also
fold the +-5/10 compat scales; the dx part of the tap offset is applied in
the product-mul read APs; the dy part is applied by the stationary of the
PE matmuls that accumulate all tap products into one fp32 PSUM accumulator
per output (multi-slice stride-0-out matmuls, start/stop flags).  All DVE
tensor ops keep 16-bit operands at even element offsets so the 2x_1p perf
mode engages (odd offsets measured ~6x slower); odd-aligned taps read from
one-cell-shifted twin tensors produced by Act copies (alignment-agnostic).
"""

import math

import numpy as np

B, C, H, W, D = 4, 21, 128, 128, 5
EPS = 1e-5
HP = 64
HE = HP + 4      # x/s row extent (halo 2)
FE = HP + 8      # feats row extent (halo 4)
BIGPAD = 100.0   # fp16-safe: 5*(100+6)^2 < 65504; exp(-0.5*s) underflows to 0
CP, CN = 10.0, 5.0  # COMPAT_PAIR, COMPAT_CLSBD

DYS = [-2, -1, 0, 1, 2]
POS = {-2: 0, 0: 1, 2: 2, -1: 3, 1: 4}   # in-group slot order: evens, odds
GI = {dy: i for i, dy in enumerate(DYS)}


def slot(dx, dy):
    return 5 * GI[dy] + POS[dx]


DIRTAPS = [(dx, dy) for dx in range(-2, 3) for dy in range(-2, 3)
           if (dx, dy) > (0, 0)]
MIRTAPS = [(dx, dy) for dx in range(-2, 3) for dy in range(-2, 3)
           if (dx, dy) < (0, 0)]

# g1full slot order: directs grouped so mirror matmul reads are affine runs
G1ORD = [(0, 2), (1, 2), (2, 2), (0, 1), (1, 1), (2, 1),
         (1, -1), (2, -1), (1, -2), (2, -2), (1, 0), (2, 0)]
G1SLOT = {t: i for i, t in enumerate(G1ORD)}

RING1 = [(-1, -1), (-1, 0), (-1, 1), (0, -1), (0, 1), (1, -1), (1, 0), (1, 1)]
RING2 = [(-2, -2), (-2, -1), (-2, 0), (-2, 1), (-2, 2), (-1, -2), (-1, 2),
         (0, -2), (0, 2), (1, -2), (1, 2), (2, -2), (2, -1), (2, 0), (2, 1),
         (2, 2)]
EXP1 = [0, 0, 1, 2, 2, 0, 2, 3, 4, 5, 7, 5, 5, 6, 7, 7]
EXP2 = [0, 1, 1, 1, 2, 3, 4, 3, 4, 3, 4, 5, 6, 6, 6, 7]
# r1 stack order: ring1 taps grouped by dy for affine builds
R1ORD = [(-1, -1), (0, -1), (1, -1), (-1, 0), (1, 0), (-1, 1), (0, 1), (1, 1)]
R1MAP = [R1ORD.index(t) for t in RING1]
# fm stack order: ring2 taps sorted by g2-stack slot
FMORD = sorted(range(16), key=lambda k: slot(*RING2[k]))
FMJ = {k: j for j, k in enumerate(FMORD)}

# (dy, out) -> (mul_engine, pair_engine|None); 'v' DVE, 'g' GpSimd;
# pair None = V0 (PE accumulates 5 raw slices)
GROUPS = {(dy, o): ['v', 'v'] for dy in DYS for o in (0, 1)}
GROUPS[(-2, 0)] = ['g', 'g']
GROUPS[(-2, 1)] = ['v', 'g']
GROUPS[(-1, 0)] = ['v', 'g']

BANKS = [(0, 512), (512, 1024), (1024, C * HP)]

_cache = {}


def _runs(pairs):
    """[(dst, src), ...] (dst-sorted) -> [(dst0, src0, ddst, dsrc, n)]."""
    out = []
    i = 0
    while i < len(pairs):
        if i + 1 < len(pairs):
            dd = pairs[i + 1][0] - pairs[i][0]
            ds = pairs[i + 1][1] - pairs[i][1]
            j = i + 1
            while (j + 1 < len(pairs)
                   and pairs[j + 1][0] - pairs[j][0] == dd
                   and pairs[j + 1][1] - pairs[j][1] == ds):
                j += 1
            out.append((pairs[i][0], pairs[i][1], dd, ds, j - i + 1))
            i = j + 1
        else:
            out.append((pairs[i][0], pairs[i][1], 1, 1, 1))
            i += 1
    return out


def _build():
    import concourse.ap as cap
    import concourse.bacc as bacc
    import concourse.mybir as mybir
    from concourse.tile import TileContext

    f16 = mybir.dt.float16
    f32 = mybir.dt.float32
    Act = mybir.ActivationFunctionType
    Alu = mybir.AluOpType

    import os
    dbg = bool(os.environ.get("KDBG"))
    nc = bacc.Bacc()
    x_d = nc.declare_dram_parameter("x", [W, C, HE], f32, isOutput=False)
    f_d = nc.declare_dram_parameter("f", [W, D, FE], f16, isOutput=False)
    s_d = nc.declare_dram_parameter("s", [W, HE], f32, isOutput=False)
    o_d = nc.declare_dram_parameter("out", [2, W, C, HP], f32, isOutput=True)
    if dbg:
        dbg_d = {nm: nc.declare_dram_parameter(nm, shp, f16, isOutput=True)
                 for nm, shp in ()}
        dbg_d["d_g2s"] = nc.declare_dram_parameter(
            "d_g2s", [W, 25, HP], f32, isOutput=True)
        dbg_d |= {nm: nc.declare_dram_parameter(nm, shp, f16, isOutput=True)
                 for nm, shp in (("d_g1s", [W, 25, HP]),
                                 
                                 ("d_xp", [W, C, HE]),
                                 ("d_w", [2, 5, W, 5, HP]),
                                 ("d_u", [10, W, 5, C, HP]))}

    def apv(ap, off, dims):
        return cap.AP(ap.tensor, ap.offset + off, [list(ap.ap[0])] + dims)

    # ---- constants (pre-TileContext, barrier-covered) ----
    def _shiftmat(name, d, val):
        t = nc.alloc_sbuf_tensor(name, [W, W], f16)
        nc.gpsimd.memset(t.ap(), 0.0)
        nc.gpsimd.affine_select(
            out=t.ap(), in_=t.ap(), compare_op=mybir.AluOpType.not_equal,
            fill=val, base=-d, pattern=[[-1, W]], channel_multiplier=1)
        return t.ap()

    s1 = {d: _shiftmat(f"s1_{d}", d, 1.0) for d in DYS}
    m5 = {d: _shiftmat(f"m5_{d}", d, -CN) for d in DYS}
    x10 = {d: _shiftmat(f"x10_{d}", d, CP) for d in DYS}
    bpad = nc.alloc_sbuf_tensor("bpad", [W, D, FE], f16)
    nc.gpsimd.memset(bpad.ap(), BIGPAD)
    zs = nc.alloc_sbuf_tensor("zs", [W, HE], f32)
    nc.gpsimd.memset(zs.ap(), 0.0)
    for cname, cval in (("c_eps", EPS), ("c_1eps", 1.0 + EPS), ("c_z", 0.0)):
        ct = nc.alloc_sbuf_tensor(cname, [W, 1], f32)
        nc.gpsimd.memset(ct.ap(), cval)
        nc.const_aps.aps[(f32, cval)] = ct.ap()
    nc.all_engine_barrier()

    with TileContext(nc) as tc:
        with (
            tc.tile_pool(name="io", bufs=1) as io,
            tc.tile_pool(name="up", bufs=3) as up,
            tc.tile_pool(name="tp", bufs=3) as tp,
            tc.tile_pool(name="scr", bufs=2) as scr,
            tc.tile_pool(name="pacc", bufs=1, space="PSUM") as pacc,
            tc.tile_pool(name="pw", bufs=2, space="PSUM") as pw,
        ):
            # ---- loads ----
            x32 = io.tile([W, C, HE], f32, tag="x32")
            f_s = {0: io.tile([W, D, FE], f16, name="f0", tag="f0")}
            s_t = {0: io.tile([W, HE], f32, name="s0", tag="s0")}
            nc.sync.dma_start(out=x32[:], in_=x_d[:])
            nc.sync.dma_start(out=f_s[0][:], in_=f_d[:])
            nc.sync.dma_start(out=s_t[0][:], in_=s_d[:])

            def _shift_load(tag, shape, dram, padsrc, dy, dt_=f16):
                t = io.tile(shape, dt_, name=tag, tag=tag)
                a, b = max(0, -dy), W - max(0, dy)
                if a > 0:
                    nc.sync.dma_start(out=t[:a], in_=padsrc[:a])
                if b < W:
                    nc.sync.dma_start(out=t[b:], in_=padsrc[b:])
                nc.sync.dma_start(out=t[a:b], in_=dram[a + dy:b + dy])
                return t

            for dy in (-2, -1, 1, 2):
                f_s[dy] = _shift_load(f"fs{dy}", [W, D, FE], f_d, bpad, dy)
                s_t[dy] = _shift_load(f"ss{dy}", [W, HE], s_d,
                                      zs[:, :HE], dy, f32)
            # odd twins (one-cell h shift) so fp16 reads stay even-aligned;
            # Act copies: alignment-agnostic engine
            f_o = {}
            for dy in DYS:
                f_o[dy] = io.tile([W, D, FE], f16, name=f"fo{dy}", tag=f"fo{dy}")
                nc.scalar.activation(f_o[dy][:, :, 0:FE - 1],
                                     f_s[dy][:, :, 1:FE], Act.Copy)

            # ---- polarness -> xp (fp16) ----
            x16 = io.tile([W, C, HE], f16, tag="x16")
            nc.scalar.activation(x16[:], x32[:], Act.Copy)
            lnx = io.tile([W, C, HE], f16, tag="lnx")
            nc.scalar.activation(lnx[:], x32[:], Act.Ln, bias=EPS)
            xl = io.tile([W, C, HE], f16, tag="xl")
            nc.vector.tensor_tensor(out=xl[:], in0=x16[:], in1=lnx[:],
                                    op=Alu.mult)
            e10 = scr.tile([W, 10, HE], f16, tag="e10")
            nc.vector.tensor_tensor(out=e10[:], in0=xl[:, 0:10],
                                    in1=xl[:, 10:20], op=Alu.add)
            e5 = scr.tile([W, 5, HE], f16, tag="e5")
            nc.vector.tensor_tensor(out=e5[:], in0=e10[:, 0:5],
                                    in1=e10[:, 5:10], op=Alu.add)
            e2 = scr.tile([W, 2, HE], f16, tag="e2")
            nc.vector.tensor_tensor(out=e2[:], in0=e5[:, 0:2],
                                    in1=e5[:, 2:4], op=Alu.add)
            e1 = scr.tile([W, 2, HE], f16, tag="e1")
            nc.vector.tensor_tensor(out=e1[:, 0], in0=e2[:, 0], in1=e2[:, 1],
                                    op=Alu.add)
            nc.vector.tensor_tensor(out=e1[:, 1], in0=e5[:, 4], in1=xl[:, 20],
                                    op=Alu.add)
            ent = scr.tile([W, HE], f16, tag="ent")
            nc.vector.tensor_tensor(out=ent[:], in0=e1[:, 0], in1=e1[:, 1],
                                    op=Alu.add)
            pl = io.tile([W, HE], f16, tag="pl")
            nc.vector.tensor_scalar(out=pl[:], in0=ent[:],
                                    scalar1=1.0 / math.log(C), scalar2=1.0,
                                    op0=Alu.mult, op1=Alu.add)
            xp = io.tile([W, C, HE], f16, tag="xp")
            nc.vector.tensor_tensor(
                out=xp[:], in0=x16[:],
                in1=pl[:, None, :].broadcast_to((W, C, HE)), op=Alu.mult)
            xpo = io.tile([W, C, HE], f16, tag="xpo")
            nc.scalar.activation(xpo[:, :, 0:HE - 1], xp[:, :, 1:HE],
                                 Act.Copy)

            # ---- pairwise gaussian: 12 direct taps into g1full ----
            g1full = io.tile([W, 12, HE], f16, tag="g1full")
            for (dx, dy) in DIRTAPS:
                k = G1SLOT[(dx, dy)]
                if dx % 2 == 0:
                    fb, off = f_s[dy], 2 + dx
                else:
                    fb, off = f_o[dy], 1 + dx
                diff = scr.tile([W, D, HE], f16, tag="diff")
                nc.vector.tensor_tensor(
                    out=diff[:], in0=f_s[0][:, :, 2:2 + HE],
                    in1=fb[:, :, off:off + HE], op=Alu.subtract)
                sq = scr.tile([W, D, HE], f16, tag="sq")
                nc.scalar.activation(sq[:], diff[:], Act.Square)
                d2 = scr.tile([W, 2, HE], f16, tag="d2")
                nc.vector.tensor_tensor(out=d2[:], in0=sq[:, 0:2],
                                        in1=sq[:, 2:4], op=Alu.add)
                d1 = scr.tile([W, HE], f16, tag="d1")
                nc.vector.tensor_tensor(out=d1[:], in0=d2[:, 0], in1=d2[:, 1],
                                        op=Alu.add)
                ssum = scr.tile([W, HE], f16, tag="ssum")
                nc.vector.tensor_tensor(out=ssum[:], in0=d1[:], in1=sq[:, 4],
                                        op=Alu.add)
                nc.scalar.activation(g1full[:, k], ssum[:], Act.Exp,
                                     scale=-0.5)

            # ---- g1 stack [W, 25, 64] (plain g1; x10 folded in PE stat) ----
            g1s = io.tile([W, 25, HP], f16, tag="g1s")
            nc.gpsimd.memset(g1s[:, slot(0, 0)], 1.0)
            g1f_f = g1full[:].rearrange("p k h -> p (k h)")
            g1s_f = g1s[:].rearrange("p k h -> p (k h)")
            dir_pairs = sorted((slot(dx, dy), G1SLOT[(dx, dy)] * HE + 2)
                               for (dx, dy) in DIRTAPS)
            for (d0, sr0, dd, ds, n) in _runs(dir_pairs):
                nc.vector.tensor_copy(
                    out=apv(g1s_f, d0 * HP, [[dd * HP, n], [1, HP]]),
                    in_=apv(g1f_f, sr0, [[ds, n], [1, HP]]))
            for dym in (-2, -1, 1, 2):
                mirs = sorted(
                    (slot(dx, dym), G1SLOT[(-dx, -dym)] * HE + 2 + dx)
                    for (dx, d2_) in MIRTAPS if d2_ == dym)
                mp = pw.tile([W, 5, HP], f32, tag="pw")
                mpf = mp[:].rearrange("p k h -> p (k h)")
                col = 0
                segs = []
                for (d0, sr0, dd, ds, n) in _runs(mirs):
                    nc.tensor.matmul(
                        mpf[:, col:col + n * HP], s1[dym],
                        apv(g1f_f, sr0, [[ds, n], [1, HP]]),
                        start=True, stop=True)
                    segs.append((d0, dd, n, col))
                    col += n * HP
                for (d0, dd, n, c0) in segs:
                    nc.scalar.activation(
                        apv(g1s_f, d0 * HP, [[dd * HP, n], [1, HP]]),
                        mpf[:, c0:c0 + n * HP], Act.Copy)
            for dx in (-1, -2):
                nc.vector.tensor_copy(
                    out=g1s[:, slot(dx, 0)],
                    in_=g1full[:, G1SLOT[(-dx, 0)], 2 + dx:2 + dx + HP])

            # ---- clsbd ring max -> g2 stack ----
            def s_ap(dx, dy, n=1, stride=1):
                return apv(s_t[dy][:], 2 + dx, [[stride, n], [1, HP]])

            r1 = io.tile([W, 8, HP], f32, tag="r1")
            for gdy, base_j in ((-1, 0), (0, 3), (1, 5)):
                taps = [t for t in R1ORD if t[1] == gdy]
                odds = [t for t in taps if t[0] % 2]
                evens = [t for t in taps if t[0] % 2 == 0]
                if odds:
                    js = [R1ORD.index(t) for t in odds]
                    st = js[1] - js[0] if len(js) > 1 else 1
                    nc.vector.tensor_copy(
                        out=apv(r1[:].rearrange("p k h -> p (k h)"),
                                js[0] * HP, [[st * HP, len(js)], [1, HP]]),
                        in_=s_ap(odds[0][0], gdy, len(odds),
                                 odds[1][0] - odds[0][0] if len(odds) > 1
                                 else 1))
                for t_ in evens:
                    nc.vector.tensor_copy(out=r1[:, R1ORD.index(t_)],
                                          in_=s_ap(t_[0], gdy))

            fm = io.tile([W, 16, HP], f32, tag="fm")
            for j, k in enumerate(FMORD):
                nc.vector.tensor_tensor(
                    out=fm[:, j], in0=r1[:, R1MAP[EXP1[k]]],
                    in1=r1[:, R1MAP[EXP2[k]]], op=Alu.max)

            g2s = io.tile([W, 25, HP], f32, tag="g2s")
            g2s_f = g2s[:].rearrange("p k h -> p (k h)")
            nc.gpsimd.memset(g2s[:, slot(0, 0)], 0.0)
            # ring1 values into stack (grouped: odd-dx pairs + singles)
            for dy in (-1, 0, 1):
                taps = [t for t in RING1 if t[1] == dy]
                odds = sorted([t for t in taps if t[0] % 2],
                              key=lambda t: slot(*t))
                evens = [t for t in taps if t[0] % 2 == 0]
                if odds:
                    sl = [slot(*t) for t in odds]
                    dxs = [t[0] for t in odds]
                    nc.vector.tensor_copy(
                        out=apv(g2s_f, sl[0] * HP,
                                [[(sl[1] - sl[0]) * HP if len(sl) > 1
                                  else HP, len(sl)], [1, HP]]),
                        in_=s_ap(dxs[0], dy, len(dxs),
                                 dxs[1] - dxs[0] if len(dxs) > 1 else 1))
                for t_ in evens:
                    nc.vector.tensor_copy(out=g2s[:, slot(*t_)],
                                          in_=s_ap(t_[0], dy))
            # ring2 second-max, grouped by dy and parity
            for dy in DYS:
                taps = [(dx, d2_) for (dx, d2_) in RING2 if d2_ == dy]
                for par in (0, 1):
                    grp = sorted([t for t in taps if abs(t[0]) % 2 == par],
                                 key=lambda t: slot(*t))
                    if not grp:
                        continue
                    sl = [slot(*t) for t in grp]
                    dxs = [t[0] for t in grp]
                    js = [FMJ[RING2.index(t)] for t in grp]
                    n = len(grp)
                    slst = sl[1] - sl[0] if n > 1 else 1
                    dxst = dxs[1] - dxs[0] if n > 1 else 1
                    jst = js[1] - js[0] if n > 1 else 1
                    nc.vector.tensor_tensor(
                        out=apv(g2s_f, sl[0] * HP, [[slst * HP, n], [1, HP]]),
                        in0=apv(fm[:].rearrange("p k h -> p (k h)"),
                                js[0] * HP, [[jst * HP, n], [1, HP]]),
                        in1=s_ap(dxs[0], dy, n, dxst), op=Alu.max)

            # ---- weights: Ln stacks, then PE shift+scale per dy ----
            lnn = io.tile([W, 25, HP], f16, tag="lnn")
            nc.scalar.activation(lnn[:], g2s[:], Act.Ln, bias=EPS)
            lnp = io.tile([W, 25, HP], f16, tag="lnp")
            nc.scalar.activation(lnp[:], g2s[:], Act.Ln, bias=1.0 + EPS,
                                 scale=-1.0)
            wsh = {}
            for dy in DYS:
                g0 = 5 * GI[dy] * HP
                pn = pw.tile([W, 5, HP], f32, tag="pw")
                pnf = pn[:].rearrange("p k h -> p (k h)")
                # wsh[m] = wn[m - dy]  ->  stationary shift by -dy
                nc.tensor.matmul(pnf[:], m5[-dy],
                                 apv(lnn[:].rearrange("p k h -> p (k h)"),
                                     g0, [[1, 5 * HP]]),
                                 start=True, stop=False)
                nc.tensor.matmul(pnf[:], x10[-dy],
                                 apv(g1s_f, g0, [[1, 5 * HP]]),
                                 start=False, stop=True)
                wn = io.tile([W, 5, HP], f16, tag=f"wn{dy}")
                nc.scalar.activation(wn[:], pn[:], Act.Copy)
                pp = pw.tile([W, 5, HP], f32, tag="pw")
                ppf = pp[:].rearrange("p k h -> p (k h)")
                nc.tensor.matmul(ppf[:], m5[-dy],
                                 apv(lnp[:].rearrange("p k h -> p (k h)"),
                                     g0, [[1, 5 * HP]]),
                                 start=True, stop=True)
                wp = io.tile([W, 5, HP], f16, tag=f"wp{dy}")
                nc.scalar.activation(wp[:], pp[:], Act.Copy)
                wsh[(dy, 0)], wsh[(dy, 1)] = wn, wp
                if dbg:
                    nc.sync.dma_start(out=dbg_d["d_w"][0, GI[dy]], in_=wn[:])
                    nc.sync.dma_start(out=dbg_d["d_w"][1, GI[dy]], in_=wp[:])

            # ---- products + PSUM accumulation ----
            acc = [pacc.tile([W, C, HP], f32, name=f"acc{o}", tag=f"acc{o}")
                   for o in (0, 1)]
            accf = [a[:].rearrange("p c h -> p (c h)") for a in acc]
            eng = {'v': nc.vector, 'g': nc.gpsimd}
            for gi_, dy in enumerate(DYS):
                for o in (0, 1):
                    me, pe_ = GROUPS[(dy, o)]
                    w = wsh[(dy, o)]
                    wf = w[:].rearrange("p k h -> p (k h)")
                    u = up.tile([W, 5, C, HP], f16, tag="u")
                    nc_m = eng[me]
                    nc_m.tensor_tensor(
                        out=u[:, 0:3],
                        in0=apv(xp[:], 0, [[2, 3], [HE, C], [1, HP]]),
                        in1=apv(wf, 0, [[HP, 3], [0, C], [1, HP]]),
                        op=Alu.mult)
                    nc_m.tensor_tensor(
                        out=u[:, 3:5],
                        in0=apv(xpo[:], 0, [[2, 2], [HE, C], [1, HP]]),
                        in1=apv(wf, 3 * HP, [[HP, 2], [0, C], [1, HP]]),
                        op=Alu.mult)
                    if dbg:
                        nc.sync.dma_start(out=dbg_d["d_u"][2 * gi_ + o],
                                          in_=u[:])
                    uf = u[:].rearrange("p s c h -> p s (c h)")
                    first, last = gi_ == 0, gi_ == len(DYS) - 1
                    if pe_ is not None:
                        t = tp.tile([W, 2, C, HP], f16, tag="t")
                        eng[pe_].tensor_tensor(
                            out=t[:], in0=u[:, 0:4:2], in1=u[:, 1:4:2],
                            op=Alu.add)
                        tf = t[:].rearrange("p s c h -> p s (c h)")
                        slices = [(tf, 2), (uf, 1)]
                    else:
                        slices = [(uf[:, 0:4], 4), (uf, 1)]
                    for (n0, n1) in BANKS:
                        for si, (sv, ns) in enumerate(slices):
                            sub = max(1, 512 // ns) if ns > 1 else 512
                            for m0 in range(n0, n1, sub):
                                m1 = min(m0 + sub, n1)
                                if ns == 1:
                                    mv = sv[:, 4, m0:m1]
                                    ov = accf[o][:, m0:m1]
                                else:
                                    mv = sv[:, 0:ns, m0:m1]
                                    ov = accf[o][:, None, m0:m1] \
                                        .broadcast_to((W, ns, m1 - m0))
                                st = first and si == 0 and m0 == n0
                                sp = last and si == len(slices) - 1 \
                                    and m1 == n1
                                nc.tensor.matmul(ov, s1[dy], mv,
                                                 start=st, stop=sp)
            if dbg:
                nc.sync.dma_start(out=dbg_d["d_g1s"][:], in_=g1s[:])
                nc.sync.dma_start(out=dbg_d["d_g2s"][:], in_=g2s[:])
                nc.sync.dma_start(out=dbg_d["d_xp"][:], in_=xp[:])
            res0 = io.tile([W, C, HP], f32, tag="res0")
            nc.scalar.activation(res0[:], acc[0][:], Act.Copy)
            res1 = io.tile([W, C, HP], f32, tag="res1")
            nc.vector.tensor_copy(out=res1[:], in_=acc[1][:])
            nc.sync.dma_start(out=o_d[0], in_=res0[:])
            nc.sync.dma_start(out=o_d[1], in_=res1[:])
    nc.finalize()
    return nc


_last_results = None


def kernel(input, feats, clsbd_feats, label=None, **_ignored):
    global _last_results
    from concourse.bass_utils import run_bass_kernel_spmd

    x = np.asarray(input, np.float32)
    f = np.asarray(feats, np.float32)
    s = np.asarray(clsbd_feats, np.float32)

    xpad = np.zeros((B, C, H + 4, W), np.float32)
    xpad[:, :, 2:2 + H] = x
    fpad = np.full((B, D, H + 8, W), BIGPAD, np.float16)
    fpad[:, :, 4:4 + H] = f.astype(np.float16)
    spad = np.zeros((B, H + 4, W), np.float32)
    spad[:, 2:2 + H] = s[:, 0]

    in_maps = []
    for i in range(8):
        b, half = i // 2, i % 2
        h0 = half * HP
        in_maps.append({
            "x": np.ascontiguousarray(
                xpad[b, :, h0:h0 + HE].transpose(2, 0, 1)),
            "f": np.ascontiguousarray(
                fpad[b, :, h0:h0 + FE].transpose(2, 0, 1)),
            "s": np.ascontiguousarray(spad[b, h0:h0 + HE].transpose(1, 0)),
        })

    if "nc" not in _cache:
        _cache["nc"] = _build()
    res = run_bass_kernel_spmd(_cache["nc"], in_maps, list(range(8)))
    _last_results = res

    out = np.empty((2, B, C, H, W), np.float32)
    for i in range(8):
        b, half = i // 2, i % 2
        h0 = half * HP
        out[:, b, :, h0:h0 + HP] = res.results[i]["out"].transpose(0, 2, 3, 1)
    return out


# revision 15
# speedup vs baseline: 2.4283x; 1.2829x over previous
"""ClsbdCRF message passing on 8 NeuronCores — weight-shift formulation.

Core i handles batch i//2, image-row half i%2 (64 output rows + halo).
Layout: W=128 on partitions, (slots, C, H) on free dims, fp16 compute with
fp32 PSUM accumulation.

msg[p] = sum_t w_t[p] * xp[p + d_t] is re-associated as
u_t[p'] = w_t[p' - d_t] * xp[p']: weights are shifted in COMPACT
[W, 5, 64] per-dy space by PE matmuls whose stationary shift matrices also
fold the +-5/10 compat scales; the dx part of the tap offset is applied in
the product-mul read APs; the dy part is applied by the stationary of the
PE matmuls that accumulate all tap products into one fp32 PSUM accumulator
per output (multi-slice stride-0-out matmuls, start/stop flags).  All DVE
tensor ops keep 16-bit operands at even element offsets so the 2x_1p perf
mode engages (odd offsets measured ~6x slower); odd-aligned taps read from
one-cell-shifted twin tensors produced by Act copies (alignment-agnostic).
"""

import math

import numpy as np

B, C, H, W, D = 4, 21, 128, 128, 5
EPS = 1e-5
HP = 64
HE = HP + 4      # x/s row extent (halo 2)
FE = HP + 8      # feats row extent (halo 4)
BIGPAD = 100.0   # fp16-safe: 5*(100+6)^2 < 65504; exp(-0.5*s) underflows to 0
CP, CN = 10.0, 5.0  # COMPAT_PAIR, COMPAT_CLSBD

DYS = [-2, -1, 0, 1, 2]
POS = {-2: 0, 0: 1, 2: 2, -1: 3, 1: 4}   # in-group slot order: evens, odds
GI = {dy: i for i, dy in enumerate(DYS)}


def slot(dx, dy):
    return 5 * GI[dy] + POS[dx]


DIRTAPS = [(dx, dy) for dx in range(-2, 3) for dy in range(-2, 3)
           if (dx, dy) > (0, 0)]
MIRTAPS = [(dx, dy) for dx in range(-2, 3) for dy in range(-2, 3)
           if (dx, dy) < (0, 0)]

# g1full slot order: directs grouped so mirror matmul reads are affine runs
G1ORD = [(0, 2), (1, 2), (2, 2), (0, 1), (1, 1), (2, 1),
         (1, -1), (2, -1), (1, -2), (2, -2), (1, 0), (2, 0)]
G1SLOT = {t: i for i, t in enumerate(G1ORD)}

RING1 = [(-1, -1), (-1, 0), (-1, 1), (0, -1), (0, 1), (1, -1), (1, 0), (1, 1)]
RING2 = [(-2, -2), (-2, -1), (-2, 0), (-2, 1), (-2, 2), (-1, -2), (-1, 2),
         (0, -2), (0, 2), (1, -2), (1, 2), (2, -2), (2, -1), (2, 0), (2, 1),
         (2, 2)]
EXP1 = [0, 0, 1, 2, 2, 0, 2, 3, 4, 5, 7, 5, 5, 6, 7, 7]
EXP2 = [0, 1, 1, 1, 2, 3, 4, 3, 4, 3, 4, 5, 6, 6, 6, 7]
# r1 stack order: ring1 taps grouped by dy for affine builds
R1ORD = [(-1, -1), (0, -1), (1, -1), (-1, 0), (1, 0), (-1, 1), (0, 1), (1, 1)]
R1MAP = [R1ORD.index(t) for t in RING1]
# fm stack order: ring2 taps sorted by g2-stack slot
FMORD = sorted(range(16), key=lambda k: slot(*RING2[k]))
FMJ = {k: j for j, k in enumerate(FMORD)}

# (dy, out) -> (mul_engine, pair_engine|None); 'v' DVE, 'g' GpSimd;
# pair None = V0 (PE accumulates 5 raw slices)
GROUPS = {(dy, o): ['v', 'v'] for dy in DYS for o in (0, 1)}
for _dy in (-2, -1):
    for _o in (0, 1):
        GROUPS[(_dy, _o)] = ['v', None]

BANKS = [(0, 512), (512, 1024), (1024, C * HP)]

_cache = {}


def _runs(pairs):
    """[(dst, src), ...] (dst-sorted) -> [(dst0, src0, ddst, dsrc, n)]."""
    out = []
    i = 0
    while i < len(pairs):
        if i + 1 < len(pairs):
            dd = pairs[i + 1][0] - pairs[i][0]
            ds = pairs[i + 1][1] - pairs[i][1]
            j = i + 1
            while (j + 1 < len(pairs)
                   and pairs[j + 1][0] - pairs[j][0] == dd
                   and pairs[j + 1][1] - pairs[j][1] == ds):
                j += 1
            out.append((pairs[i][0], pairs[i][1], dd, ds, j - i + 1))
            i = j + 1
        else:
            out.append((pairs[i][0], pairs[i][1], 1, 1, 1))
            i += 1
    return out


def _build():
    import concourse.ap as cap
    import concourse.bacc as bacc
    import concourse.mybir as mybir
    from concourse.tile import TileContext

    f16 = mybir.dt.float16
    f32 = mybir.dt.float32
    Act = mybir.ActivationFunctionType
    Alu = mybir.AluOpType

    import os
    dbg = bool(os.environ.get("KDBG"))
    nc = bacc.Bacc()
    x_d = nc.declare_dram_parameter("x", [W, C, HE], f32, isOutput=False)
    f_d = nc.declare_dram_parameter("f", [W, D, FE], f16, isOutput=False)
    s_d = nc.declare_dram_parameter("s", [W, HE], f32, isOutput=False)
    o_d = nc.declare_dram_parameter("out", [2, W, C, HP], f32, isOutput=True)
    if dbg:
        dbg_d = {nm: nc.declare_dram_parameter(nm, shp, f16, isOutput=True)
                 for nm, shp in ()}
        dbg_d["d_g2s"] = nc.declare_dram_parameter(
            "d_g2s", [W, 25, HP], f32, isOutput=True)
        dbg_d |= {nm: nc.declare_dram_parameter(nm, shp, f16, isOutput=True)
                 for nm, shp in (("d_g1s", [W, 25, HP]),
                                 
                                 ("d_xp", [W, C, HE]),
                                 ("d_w", [2, 5, W, 5, HP]),
                                 ("d_u", [10, W, 5, C, HP]))}

    def apv(ap, off, dims):
        return cap.AP(ap.tensor, ap.offset + off, [list(ap.ap[0])] + dims)

    # ---- constants (pre-TileContext, barrier-covered) ----
    def _shiftmat(name, d, val):
        t = nc.alloc_sbuf_tensor(name, [W, W], f16)
        nc.gpsimd.memset(t.ap(), 0.0)
        nc.gpsimd.affine_select(
            out=t.ap(), in_=t.ap(), compare_op=mybir.AluOpType.not_equal,
            fill=val, base=-d, pattern=[[-1, W]], channel_multiplier=1)
        return t.ap()

    s1 = {d: _shiftmat(f"s1_{d}", d, 1.0) for d in DYS}
    m5 = {d: _shiftmat(f"m5_{d}", d, -CN) for d in DYS}
    x10 = {d: _shiftmat(f"x10_{d}", d, CP) for d in DYS}
    bpad = nc.alloc_sbuf_tensor("bpad", [W, D, FE], f16)
    nc.gpsimd.memset(bpad.ap(), BIGPAD)
    zs = nc.alloc_sbuf_tensor("zs", [W, HE], f32)
    nc.gpsimd.memset(zs.ap(), 0.0)
    for cname, cval in (("c_eps", EPS), ("c_1eps", 1.0 + EPS), ("c_z", 0.0)):
        ct = nc.alloc_sbuf_tensor(cname, [W, 1], f32)
        nc.gpsimd.memset(ct.ap(), cval)
        nc.const_aps.aps[(f32, cval)] = ct.ap()
    nc.all_engine_barrier()

    with TileContext(nc) as tc:
        with (
            tc.tile_pool(name="io", bufs=1) as io,
            tc.tile_pool(name="up", bufs=3) as up,
            tc.tile_pool(name="tp", bufs=3) as tp,
            tc.tile_pool(name="scr", bufs=2) as scr,
            tc.tile_pool(name="pacc", bufs=1, space="PSUM") as pacc,
            tc.tile_pool(name="pw", bufs=2, space="PSUM") as pw,
        ):
            # ---- loads: s first (rings are the earliest DVE work) ----
            s_t = {0: io.tile([W, HE], f32, name="s0", tag="s0")}
            nc.sync.dma_start(out=s_t[0][:], in_=s_d[:])
            x32 = io.tile([W, C, HE], f32, tag="x32")
            f_s = {0: io.tile([W, D, FE], f16, name="f0", tag="f0")}
            nc.sync.dma_start(out=x32[:], in_=x_d[:])
            nc.sync.dma_start(out=f_s[0][:], in_=f_d[:])

            def _shift_load(tag, shape, dram, padsrc, dy, dt_, q):
                t = io.tile(shape, dt_, name=tag, tag=tag)
                a, b = max(0, -dy), W - max(0, dy)
                if a > 0:
                    q.dma_start(out=t[:a], in_=padsrc[:a])
                if b < W:
                    q.dma_start(out=t[b:], in_=padsrc[b:])
                q.dma_start(out=t[a:b], in_=dram[a + dy:b + dy])
                return t

            for dy in (-2, -1, 1, 2):
                s_t[dy] = _shift_load(f"ss{dy}", [W, HE], s_d,
                                      zs[:, :HE], dy, f32, nc.sync)
            for dy in (-2, -1, 1, 2):
                f_s[dy] = _shift_load(f"fs{dy}", [W, D, FE], f_d, bpad, dy,
                                      f16, nc.sync)

            # ---- clsbd ring max -> g2 stack (fp32, DVE; earliest work) ----
            def s_ap(dx, dy, n=1, stride=1):
                return apv(s_t[dy][:], 2 + dx, [[stride, n], [1, HP]])

            r1 = io.tile([W, 8, HP], f32, tag="r1")
            for gdy in (-1, 0, 1):
                taps = [t for t in R1ORD if t[1] == gdy]
                odds = [t for t in taps if t[0] % 2]
                evens = [t for t in taps if t[0] % 2 == 0]
                if odds:
                    js = [R1ORD.index(t) for t in odds]
                    st = js[1] - js[0] if len(js) > 1 else 1
                    nc.vector.tensor_copy(
                        out=apv(r1[:].rearrange("p k h -> p (k h)"),
                                js[0] * HP, [[st * HP, len(js)], [1, HP]]),
                        in_=s_ap(odds[0][0], gdy, len(odds),
                                 odds[1][0] - odds[0][0] if len(odds) > 1
                                 else 1))
                for t_ in evens:
                    nc.vector.tensor_copy(out=r1[:, R1ORD.index(t_)],
                                          in_=s_ap(t_[0], gdy))

            fm = io.tile([W, 16, HP], f32, tag="fm")
            for j, k in enumerate(FMORD):
                nc.vector.tensor_tensor(
                    out=fm[:, j], in0=r1[:, R1MAP[EXP1[k]]],
                    in1=r1[:, R1MAP[EXP2[k]]], op=Alu.max)

            g2s = io.tile([W, 25, HP], f32, tag="g2s")
            g2s_f = g2s[:].rearrange("p k h -> p (k h)")
            nc.gpsimd.memset(g2s[:, slot(0, 0)], 0.0)
            for dy in (-1, 0, 1):
                taps = [t for t in RING1 if t[1] == dy]
                odds = sorted([t for t in taps if t[0] % 2],
                              key=lambda t: slot(*t))
                evens = [t for t in taps if t[0] % 2 == 0]
                if odds:
                    sl = [slot(*t) for t in odds]
                    dxs = [t[0] for t in odds]
                    nc.vector.tensor_copy(
                        out=apv(g2s_f, sl[0] * HP,
                                [[(sl[1] - sl[0]) * HP if len(sl) > 1
                                  else HP, len(sl)], [1, HP]]),
                        in_=s_ap(dxs[0], dy, len(dxs),
                                 dxs[1] - dxs[0] if len(dxs) > 1 else 1))
                for t_ in evens:
                    nc.vector.tensor_copy(out=g2s[:, slot(*t_)],
                                          in_=s_ap(t_[0], dy))
            for dy in DYS:
                taps = [(dx, d2_) for (dx, d2_) in RING2 if d2_ == dy]
                for par in (0, 1):
                    grp = sorted([t for t in taps if abs(t[0]) % 2 == par],
                                 key=lambda t: slot(*t))
                    if not grp:
                        continue
                    sl = [slot(*t) for t in grp]
                    dxs = [t[0] for t in grp]
                    js = [FMJ[RING2.index(t)] for t in grp]
                    n = len(grp)
                    slst = sl[1] - sl[0] if n > 1 else 1
                    dxst = dxs[1] - dxs[0] if n > 1 else 1
                    jst = js[1] - js[0] if n > 1 else 1
                    nc.vector.tensor_tensor(
                        out=apv(g2s_f, sl[0] * HP, [[slst * HP, n], [1, HP]]),
                        in0=apv(fm[:].rearrange("p k h -> p (k h)"),
                                js[0] * HP, [[jst * HP, n], [1, HP]]),
                        in1=s_ap(dxs[0], dy, n, dxst), op=Alu.max)

            # ---- Ln cluster (one act-table residency) ----
            lnx = io.tile([W, C, HE], f16, tag="lnx")
            nc.scalar.activation(lnx[:], x32[:], Act.Ln, bias=EPS)
            lnn = io.tile([W, 25, HP], f16, tag="lnn")
            nc.scalar.activation(lnn[:], g2s[:], Act.Ln, bias=EPS)
            lnp = io.tile([W, 25, HP], f16, tag="lnp")
            nc.scalar.activation(lnp[:], g2s[:], Act.Ln, bias=1.0 + EPS,
                                 scale=-1.0)

            # ---- fp16 converts / odd twins (Act copies) ----
            x16 = io.tile([W, C, HE], f16, tag="x16")
            nc.scalar.activation(x16[:], x32[:], Act.Copy)
            f_o = {}
            for dy in DYS:
                f_o[dy] = io.tile([W, D, FE], f16, name=f"fo{dy}",
                                  tag=f"fo{dy}")
                nc.scalar.activation(f_o[dy][:, :, 0:FE - 1],
                                     f_s[dy][:, :, 1:FE], Act.Copy)

            # ---- polarness -> xp (DVE fp16) ----
            xl = io.tile([W, C, HE], f16, tag="xl")
            nc.vector.tensor_tensor(out=xl[:], in0=x16[:], in1=lnx[:],
                                    op=Alu.mult)
            e10 = scr.tile([W, 10, HE], f16, tag="e10")
            nc.vector.tensor_tensor(out=e10[:], in0=xl[:, 0:10],
                                    in1=xl[:, 10:20], op=Alu.add)
            e5 = scr.tile([W, 5, HE], f16, tag="e5")
            nc.vector.tensor_tensor(out=e5[:], in0=e10[:, 0:5],
                                    in1=e10[:, 5:10], op=Alu.add)
            e2 = scr.tile([W, 2, HE], f16, tag="e2")
            nc.vector.tensor_tensor(out=e2[:], in0=e5[:, 0:2],
                                    in1=e5[:, 2:4], op=Alu.add)
            e1 = scr.tile([W, 2, HE], f16, tag="e1")
            nc.vector.tensor_tensor(out=e1[:, 0], in0=e2[:, 0], in1=e2[:, 1],
                                    op=Alu.add)
            nc.vector.tensor_tensor(out=e1[:, 1], in0=e5[:, 4], in1=xl[:, 20],
                                    op=Alu.add)
            ent = scr.tile([W, HE], f16, tag="ent")
            nc.vector.tensor_tensor(out=ent[:], in0=e1[:, 0], in1=e1[:, 1],
                                    op=Alu.add)
            pl = io.tile([W, HE], f16, tag="pl")
            nc.vector.tensor_scalar(out=pl[:], in0=ent[:],
                                    scalar1=1.0 / math.log(C), scalar2=1.0,
                                    op0=Alu.mult, op1=Alu.add)
            xp = io.tile([W, C, HE], f16, tag="xp")
            nc.vector.tensor_tensor(
                out=xp[:], in0=x16[:],
                in1=pl[:, None, :].broadcast_to((W, C, HE)), op=Alu.mult)
            xpo = io.tile([W, C, HE], f16, tag="xpo")
            nc.scalar.activation(xpo[:, :, 0:HE - 1], xp[:, :, 1:HE],
                                 Act.Copy)

            # ---- pairwise gaussian: 12 direct taps into g1full ----
            g1full = io.tile([W, 12, HE], f16, tag="g1full")
            for (dx, dy) in DIRTAPS:
                k = G1SLOT[(dx, dy)]
                if dx % 2 == 0:
                    fb, off = f_s[dy], 2 + dx
                else:
                    fb, off = f_o[dy], 1 + dx
                diff = scr.tile([W, D, HE], f16, tag="diff")
                nc.vector.tensor_tensor(
                    out=diff[:], in0=f_s[0][:, :, 2:2 + HE],
                    in1=fb[:, :, off:off + HE], op=Alu.subtract)
                sq = scr.tile([W, D, HE], f16, tag="sq")
                nc.scalar.activation(sq[:], diff[:], Act.Square)
                d2 = scr.tile([W, 2, HE], f16, tag="d2")
                nc.vector.tensor_tensor(out=d2[:], in0=sq[:, 0:2],
                                        in1=sq[:, 2:4], op=Alu.add)
                d1 = scr.tile([W, HE], f16, tag="d1")
                nc.vector.tensor_tensor(out=d1[:], in0=d2[:, 0], in1=d2[:, 1],
                                        op=Alu.add)
                ssum = scr.tile([W, HE], f16, tag="ssum")
                nc.vector.tensor_tensor(out=ssum[:], in0=d1[:], in1=sq[:, 4],
                                        op=Alu.add)
                nc.scalar.activation(g1full[:, k], ssum[:], Act.Exp,
                                     scale=-0.5)

            # ---- g1 stack [W, 25, 64] (plain g1; x10 folded in PE stat) ----
            g1s = io.tile([W, 25, HP], f16, tag="g1s")
            nc.gpsimd.memset(g1s[:, slot(0, 0)], 1.0)
            g1f_f = g1full[:].rearrange("p k h -> p (k h)")
            g1s_f = g1s[:].rearrange("p k h -> p (k h)")
            dir_pairs = sorted((slot(dx, dy), G1SLOT[(dx, dy)] * HE + 2)
                               for (dx, dy) in DIRTAPS)
            for (d0, sr0, dd, ds, n) in _runs(dir_pairs):
                nc.vector.tensor_copy(
                    out=apv(g1s_f, d0 * HP, [[dd * HP, n], [1, HP]]),
                    in_=apv(g1f_f, sr0, [[ds, n], [1, HP]]))
            for dym in (-2, -1, 1, 2):
                mirs = sorted(
                    (slot(dx, dym), G1SLOT[(-dx, -dym)] * HE + 2 + dx)
                    for (dx, d2_) in MIRTAPS if d2_ == dym)
                mp = pw.tile([W, 5, HP], f32, tag="pw")
                mpf = mp[:].rearrange("p k h -> p (k h)")
                col = 0
                segs = []
                for (d0, sr0, dd, ds, n) in _runs(mirs):
                    nc.tensor.matmul(
                        mpf[:, col:col + n * HP], s1[dym],
                        apv(g1f_f, sr0, [[ds, n], [1, HP]]),
                        start=True, stop=True)
                    segs.append((d0, dd, n, col))
                    col += n * HP
                for (d0, dd, n, c0) in segs:
                    nc.scalar.activation(
                        apv(g1s_f, d0 * HP, [[dd * HP, n], [1, HP]]),
                        mpf[:, c0:c0 + n * HP], Act.Copy)
            for dx in (-1, -2):
                nc.vector.tensor_copy(
                    out=g1s[:, slot(dx, 0)],
                    in_=g1full[:, G1SLOT[(-dx, 0)], 2 + dx:2 + dx + HP])

            # ---- weights: PE shift+scale per dy ----
            wsh = {}
            for dy in DYS:
                g0 = 5 * GI[dy] * HP
                pn = pw.tile([W, 5, HP], f32, tag="pw")
                pnf = pn[:].rearrange("p k h -> p (k h)")
                # wsh[m] = wn[m - dy]  ->  stationary shift by -dy
                nc.tensor.matmul(pnf[:], m5[-dy],
                                 apv(lnn[:].rearrange("p k h -> p (k h)"),
                                     g0, [[1, 5 * HP]]),
                                 start=True, stop=False)
                nc.tensor.matmul(pnf[:], x10[-dy],
                                 apv(g1s_f, g0, [[1, 5 * HP]]),
                                 start=False, stop=True)
                wn = io.tile([W, 5, HP], f16, name=f"wn{dy}", tag=f"wn{dy}")
                nc.scalar.activation(wn[:], pn[:], Act.Copy)
                pp = pw.tile([W, 5, HP], f32, tag="pw")
                ppf = pp[:].rearrange("p k h -> p (k h)")
                nc.tensor.matmul(ppf[:], m5[-dy],
                                 apv(lnp[:].rearrange("p k h -> p (k h)"),
                                     g0, [[1, 5 * HP]]),
                                 start=True, stop=True)
                wp = io.tile([W, 5, HP], f16, name=f"wp{dy}", tag=f"wp{dy}")
                nc.scalar.activation(wp[:], pp[:], Act.Copy)
                wsh[(dy, 0)], wsh[(dy, 1)] = wn, wp
                if dbg:
                    nc.sync.dma_start(out=dbg_d["d_w"][0, GI[dy]], in_=wn[:])
                    nc.sync.dma_start(out=dbg_d["d_w"][1, GI[dy]], in_=wp[:])

            # ---- products (DVE muls; pairs DVE or V0->PE) + accumulation ----
            acc = [pacc.tile([W, C, HP], f32, name=f"acc{o}", tag=f"acc{o}")
                   for o in (0, 1)]
            accf = [a[:].rearrange("p c h -> p (c h)") for a in acc]
            for gi_, dy in enumerate(DYS):
                for o in (0, 1):
                    _, pe_ = GROUPS[(dy, o)]
                    w = wsh[(dy, o)]
                    wf = w[:].rearrange("p k h -> p (k h)")
                    u = up.tile([W, 5, C, HP], f16, tag="u")
                    nc.vector.tensor_tensor(
                        out=u[:, 0:3],
                        in0=apv(xp[:], 0, [[2, 3], [HE, C], [1, HP]]),
                        in1=apv(wf, 0, [[HP, 3], [0, C], [1, HP]]),
                        op=Alu.mult)
                    nc.vector.tensor_tensor(
                        out=u[:, 3:5],
                        in0=apv(xpo[:], 0, [[2, 2], [HE, C], [1, HP]]),
                        in1=apv(wf, 3 * HP, [[HP, 2], [0, C], [1, HP]]),
                        op=Alu.mult)
                    if dbg:
                        nc.sync.dma_start(out=dbg_d["d_u"][2 * gi_ + o],
                                          in_=u[:])
                    uf = u[:].rearrange("p s c h -> p s (c h)")
                    first, last = gi_ == 0, gi_ == len(DYS) - 1
                    if pe_ is not None:
                        t = tp.tile([W, 2, C, HP], f16, tag="t")
                        nc.vector.tensor_tensor(
                            out=t[:], in0=u[:, 0:4:2], in1=u[:, 1:4:2],
                            op=Alu.add)
                        tf = t[:].rearrange("p s c h -> p s (c h)")
                        slices = [(tf, 2), (uf, 1)]
                    else:
                        slices = [(uf[:, 0:4], 4), (uf, 1)]
                    for (n0, n1) in BANKS:
                        for si, (sv, ns) in enumerate(slices):
                            sub = max(1, 512 // ns) if ns > 1 else 512
                            for m0 in range(n0, n1, sub):
                                m1 = min(m0 + sub, n1)
                                if ns == 1:
                                    mv = sv[:, 4, m0:m1]
                                    ov = accf[o][:, m0:m1]
                                else:
                                    mv = sv[:, 0:ns, m0:m1]
                                    ov = accf[o][:, None, m0:m1] \
                                        .broadcast_to((W, ns, m1 - m0))
                                st = first and si == 0 and m0 == n0
                                sp = last and si == len(slices) - 1 \
                                    and m1 == n1
                                nc.tensor.matmul(ov, s1[dy], mv,
                                                 start=st, stop=sp)
            if dbg:
                nc.sync.dma_start(out=dbg_d["d_g1s"][:], in_=g1s[:])
                nc.sync.dma_start(out=dbg_d["d_g2s"][:], in_=g2s[:])
                nc.sync.dma_start(out=dbg_d["d_xp"][:], in_=xp[:])
            res0 = io.tile([W, C, HP], f32, tag="res0")
            nc.scalar.activation(res0[:], acc[0][:], Act.Copy)
            res1 = io.tile([W, C, HP], f32, tag="res1")
            nc.vector.tensor_copy(out=res1[:], in_=acc[1][:])
            nc.sync.dma_start(out=o_d[0], in_=res0[:])
            nc.sync.dma_start(out=o_d[1], in_=res1[:])
    nc.finalize()
    return nc


_last_results = None


def kernel(input, feats, clsbd_feats, label=None, **_ignored):
    global _last_results
    from concourse.bass_utils import run_bass_kernel_spmd

    x = np.asarray(input, np.float32)
    f = np.asarray(feats, np.float32)
    s = np.asarray(clsbd_feats, np.float32)

    xpad = np.zeros((B, C, H + 4, W), np.float32)
    xpad[:, :, 2:2 + H] = x
    fpad = np.full((B, D, H + 8, W), BIGPAD, np.float16)
    fpad[:, :, 4:4 + H] = f.astype(np.float16)
    spad = np.zeros((B, H + 4, W), np.float32)
    spad[:, 2:2 + H] = s[:, 0]

    in_maps = []
    for i in range(8):
        b, half = i // 2, i % 2
        h0 = half * HP
        in_maps.append({
            "x": np.ascontiguousarray(
                xpad[b, :, h0:h0 + HE].transpose(2, 0, 1)),
            "f": np.ascontiguousarray(
                fpad[b, :, h0:h0 + FE].transpose(2, 0, 1)),
            "s": np.ascontiguousarray(spad[b, h0:h0 + HE].transpose(1, 0)),
        })

    if "nc" not in _cache:
        _cache["nc"] = _build()
    res = run_bass_kernel_spmd(_cache["nc"], in_maps, list(range(8)))
    _last_results = res

    out = np.empty((2, B, C, H, W), np.float32)
    for i in range(8):
        b, half = i // 2, i % 2
        h0 = half * HP
        out[:, b, :, h0:h0 + HP] = res.results[i]["out"].transpose(0, 2, 3, 1)
    return out
